# revision 33
# baseline (speedup 1.0000x reference)
"""Locally banded sparse attention (window=64) on 8 Trainium2 NeuronCores.

Sequence-parallel: each core owns 256 contiguous query positions and
receives a 384-row x chunk (its 256 rows + 64-row halo on each side,
zero-padded at the sequence edges) plus a full replica of the four
projection matrices.  No device collectives are needed.

All matmuls run in bf16 (fp32 PSUM accumulation).  Attention scores are
computed directly in transposed layout S^T[key, query] = kT.T @ qT, and
the P@V matmul uses P^T as the stationary operand so its output lands
query-major: av[q, d] with the softmax denominator Z[q] riding along as a
ones-column of V (col 64 of each head's 65-wide slot).  Normalization is
then a per-partition reciprocal + tensor_scalar multiply — no partition
broadcasts anywhere.  The q-major attention output is PE-transposed back
to d-major for the output projection.

Host-side folds: SCALE and bq into Wq/bq, bv into an effective bo
(out += bv @ Wo.T is query-independent).

Engine balance: PE matmuls (plus HAM warm-up dummies during the input
DMA); ACT exp + kT/o copies; DVE qT/vaug copies, reciprocals,
normalization, transpose copies; GPSIMD half the band-mask multiplies.

v3 skeleton: 4 consumption-ordered input DMAs [x | wk g0 | wq g0 |
biases], [wv], [wk g1-3 | wq g1-3 | mask | ident], [wo], so K/Q group-0
projections and the V projection overlap the remaining weight traffic;
group 1-3 K/Q projections and the aT transposes interleave with the
attention steps.
"""

import numpy as np
import ml_dtypes

import concourse.bass as bass
import concourse.tile as tile
from concourse import bacc, mybir
from concourse import bass_utils

F32 = mybir.dt.float32
BF16 = mybir.dt.bfloat16
N_CORES = 8
S = 2048
D = 512
H = 8
DK = 64
W = 64
SCALE = 1.0 / np.sqrt(DK)
SEQ_PER_CORE = S // N_CORES          # 256
CHUNK = SEQ_PER_CORE + 2 * W         # 384 rows of k/v context per core

_CACHE = {}


def _build_program():
    nc = bacc.Bacc("TRN2", target_bir_lowering=False, debug=False,
                   num_devices=N_CORES)

    # d1 = x (1536) | wk g0 (512) | wq g0 (512) | biases f32-bitcast (24)
    # d3 = wk g1-3 (1536) | wq g1-3 (1536) | mask (512) | ident (128)
    d1a = nc.dram_tensor("d1a", [128, 768], BF16, kind="ExternalInput").ap()
    d1a2 = nc.dram_tensor("d1a2", [128, 768], BF16, kind="ExternalInput").ap()
    d1b = nc.dram_tensor("d1b", [128, 1048], BF16, kind="ExternalInput").ap()
    wv4 = nc.dram_tensor("wv4", [128, 2048], BF16, kind="ExternalInput").ap()
    d3 = nc.dram_tensor("d3", [128, 3712], BF16, kind="ExternalInput").ap()
    wo4 = nc.dram_tensor("wo4", [128, 2048], BF16, kind="ExternalInput").ap()
    outT = nc.dram_tensor("outT", [128, 4 * SEQ_PER_CORE], BF16,
                          kind="ExternalOutput").ap()

    with tile.TileContext(nc) as tc:
        with (
            tc.tile_pool(name="const", bufs=1) as cpool,
            tc.tile_pool(name="pp", bufs=2, space="PSUM") as pp,
            tc.tile_pool(name="s_ps", bufs=4, space="PSUM") as s_ps,
            tc.tile_pool(name="av_ps", bufs=2, space="PSUM") as av_ps,
            tc.tile_pool(name="soft", bufs=6) as soft,
            tc.tile_pool(name="small", bufs=4) as small,
        ):
            def persist(shape, tag, dtype=BF16):
                return cpool.tile(shape, dtype, tag=tag, name=tag)

            d1_sb = persist([128, 2584], "d1")
            wv_sb = persist([128, 2048], "wv")
            d3_sb = persist([128, 3712], "d3")
            ident = d3_sb[:, 3584:3712]
            wo_sb = persist([128, 2048], "wo")
            x_sb = d1_sb[:, 0:1536]
            bias_sb = d1_sb[:, 2560:2584].bitcast(F32)

            def wk_slice(g, kk):
                if g == 0:
                    return d1_sb[:, 1536 + kk * 128:1536 + kk * 128 + 128]
                return d3_sb[:, (g - 1) * 512 + kk * 128:
                             (g - 1) * 512 + kk * 128 + 128]

            def wq_slice(g, kk):
                if g == 0:
                    return d1_sb[:, 2048 + kk * 128:2048 + kk * 128 + 128]
                return d3_sb[:, 1536 + (g - 1) * 512 + kk * 128:
                             1536 + (g - 1) * 512 + kk * 128 + 128]
            mask_sb = d3_sb[:, 3072:3584]
            k_sb = [persist([128, CHUNK], f"k{g}") for g in range(4)]
            q_sb = [persist([128, SEQ_PER_CORE], f"q{g}") for g in range(4)]
            # v with a ones column per head: head h at cols [h*65, +64], Z at h*65+64
            vaug = [persist([128, 8 * 65], f"v{r}") for r in range(3)]
            aT_sb = [persist([128, D], f"aT{t}") for t in range(2)]
            a_sb = [persist([128, SEQ_PER_CORE], f"a{g}") for g in range(4)]
            o_all = persist([128, 4 * SEQ_PER_CORE], "o_all")
            scratch = persist([128, 256], "scratch")

            # input DMAs: single sync HWDGE ring, strict consumption order
            # (in-ring transfers complete FIFO; each dma_start costs ~0.6us
            # of ring issue time, so inputs are merged into 4 transfers)
            nc.sync.dma_start(d1_sb[:, 0:768], d1a[:, :])
            nc.sync.dma_start(d1_sb[:, 768:1536], d1a2[:, :])
            nc.sync.dma_start(d1_sb[:, 1536:2584], d1b[:, :])
            nc.sync.dma_start(wv_sb[:], wv4[:, :])
            nc.sync.dma_start(d3_sb[:], d3[:, :])
            nc.sync.dma_start(wo_sb[:], wo4[:, :])

            def vaug_ap(r, col0, ncols):
                base = vaug[r][:]
                p_step = base.ap[0][0]
                return bass.AP(base.tensor, base.offset + col0,
                               [[p_step, 128], [65, 8], [1, ncols]])

            for r in range(3):
                nc.vector.memset(vaug_ap(r, 64, 1), 1.0)

            # HAM warm-up: keep the PE streaming dummy matmuls while the
            # weight DMAs land so real matmuls run at 2.4 GHz, not 1.2
            nc.vector.memset(scratch[:], 0.0)
            # touch the ACT table early so its 1.3us load runs during the
            # input-DMA wait, not on the K-evac critical path
            warm_act = small.tile([128, 8], BF16, tag="wa", name="wa")
            nc.scalar.activation(warm_act[:], scratch[:, 0:8],
                                 mybir.ActivationFunctionType.Exp)
            for w in range(18):
                wps = s_ps.tile([128, 256], F32, tag="s", name="warm")
                nc.tensor.matmul(wps[:], scratch[:, 0:128], scratch[:],
                                 start=True, stop=True)

            # ---- projections -----------------------------------------
            def emit_kproj(g):
                ps = pp.tile([128, 512], F32, tag="pp", name="pp")
                for kk in range(4):
                    nc.tensor.matmul(ps[:, :CHUNK],
                                     wk_slice(g, kk),
                                     x_sb[:, kk * CHUNK:(kk + 1) * CHUNK],
                                     start=(kk == 0), stop=(kk == 3))
                nc.scalar.activation(k_sb[g][:], ps[:, :CHUNK],
                                     mybir.ActivationFunctionType.Identity,
                                     bias=bias_sb[:, g:g + 1])

            def emit_qproj(g):
                ps = pp.tile([128, 512], F32, tag="pp", name="pp")
                for kk in range(4):
                    nc.tensor.matmul(ps[:, :SEQ_PER_CORE],
                                     wq_slice(g, kk),
                                     x_sb[:, kk * CHUNK + W:
                                          kk * CHUNK + W + SEQ_PER_CORE],
                                     start=(kk == 0), stop=(kk == 3))
                nc.vector.tensor_scalar_add(q_sb[g][:], ps[:, :SEQ_PER_CORE],
                                            bias_sb[:, 8 + g:9 + g])

            emit_kproj(0)
            emit_qproj(0)
            # v natural [keys, dout] -> vaug 65-wide head slots
            for r in range(3):
                ps = pp.tile([128, 512], F32, tag="pp", name="pp")
                for kk in range(4):
                    nc.tensor.matmul(ps[:],
                                     x_sb[:, kk * CHUNK + r * 128:
                                          kk * CHUNK + r * 128 + 128],
                                     wv_sb[:, kk * 512:(kk + 1) * 512],
                                     start=(kk == 0), stop=(kk == 3))
                if r % 2 == 0:
                    nc.vector.tensor_copy(vaug_ap(r, 0, 64), ps[:])
                else:
                    nc.scalar.activation(vaug_ap(r, 0, 64), ps[:],
                                         mybir.ActivationFunctionType.Copy)

            # ---- banded attention (S^T scores, q-major AV) ----------
            # software pipeline: S^T for step i runs on PE while step i-1
            # finishes softmax on ACT/GPSIMD, then its AV matmuls issue.
            steps = [(g, t) for g in range(4) for t in range(2)]
            pend = None   # (g, t, pA, pB, avz)
            tposes = []

            def emit_transpose(g, t):
                tp = av_ps.tile([128, 128], BF16, tag="av", name="tp")
                nc.tensor.transpose(tp[:],
                                    aT_sb[t][:, g * 128:(g + 1) * 128],
                                    ident)
                if g % 2 == 0:
                    nc.vector.tensor_copy(
                        a_sb[g][:, t * 128:(t + 1) * 128], tp[:])
                else:
                    nc.scalar.activation(
                        a_sb[g][:, t * 128:(t + 1) * 128], tp[:],
                        mybir.ActivationFunctionType.Copy)

            def emit_av(st):
                g, t, pA, pB, avz = st
                hA, hB = 2 * g, 2 * g + 1
                for kb in range(2):
                    nc.tensor.matmul(avz[:, 0:65],
                                     pA[:, kb * 128:(kb + 1) * 128],
                                     vaug[t + kb][:, hA * 65:hA * 65 + 65],
                                     start=(kb == 0), stop=(kb == 1))
                for kb in range(2):
                    nc.tensor.matmul(avz[:, 65:130],
                                     pB[:, kb * 128:(kb + 1) * 128],
                                     vaug[t + kb][:, hB * 65:hB * 65 + 65],
                                     start=(kb == 0), stop=(kb == 1))
                rz2 = small.tile([128, 2], F32, tag="rz", name="rz")
                zbase = avz[:]
                pstep = zbase.ap[0][0]
                zin = bass.AP(zbase.tensor, zbase.offset + 64,
                              [[pstep, 128], [65, 2]])
                nc.vector.reciprocal_approx_fast(rz2[:], zin)
                nc.vector.tensor_scalar_mul(aT_sb[t][:, hA * 64:hA * 64 + 64],
                                            avz[:, 0:64], rz2[:, 0:1])
                nc.vector.tensor_scalar_mul(aT_sb[t][:, hB * 64:hB * 64 + 64],
                                            avz[:, 65:129], rz2[:, 1:2])

            for g, t in steps:
                sA = s_ps.tile([128, 256], F32, tag="s", name="s")
                sB = s_ps.tile([128, 256], F32, tag="s", name="s")
                msk = mask_sb[:, t * 256:(t + 1) * 256]
                nc.tensor.matmul(sA[:], ident, msk, start=True, stop=False)
                for kb in range(2):
                    kc = (t + kb) * 128
                    nc.tensor.matmul(sA[:, kb * 128:(kb + 1) * 128],
                                     k_sb[g][0:64, kc:kc + 128],
                                     q_sb[g][0:64, t * 128:(t + 1) * 128],
                                     start=False, stop=(kb == 1))
                nc.tensor.matmul(sB[:], ident, msk, start=True, stop=False)
                for kb in range(2):
                    kc = (t + kb) * 128
                    nc.tensor.matmul(sB[:, kb * 128:(kb + 1) * 128],
                                     k_sb[g][64:128, kc:kc + 128],
                                     q_sb[g][64:128, t * 128:(t + 1) * 128],
                                     start=False, stop=(kb == 1))
                pA = soft.tile([128, 256], BF16, tag="pA", name="pA")
                pB = soft.tile([128, 256], BF16, tag="pB", name="pB")
                nc.scalar.activation(pA[:], sA[:],
                                     mybir.ActivationFunctionType.Exp)
                nc.scalar.activation(pB[:], sB[:],
                                     mybir.ActivationFunctionType.Exp)
                avz = av_ps.tile([128, 130], F32, tag="av", name="av")
                if pend is not None:
                    emit_av(pend)
                    tposes.append((pend[0], pend[1]))
                pend = (g, t, pA, pB, avz)
                if t == 0 and g < 3:
                    emit_kproj(g + 1)
                    emit_qproj(g + 1)
                if len(tposes) >= 1:
                    emit_transpose(*tposes.pop(0))
            emit_av(pend)
            tposes.append((pend[0], pend[1]))
            for gt in tposes:
                emit_transpose(*gt)

            # ---- O-projection (gg-outer over 4 s_ps banks) ----------
            o_ps = [s_ps.tile([128, 256], F32, tag="s", name=f"o{gg}")
                    for gg in range(4)]
            for gg in range(4):
                for g in range(4):
                    nc.tensor.matmul(o_ps[gg][:],
                                     wo_sb[:, g * 512 + gg * 128:
                                           g * 512 + gg * 128 + 128],
                                     a_sb[g][:],
                                     start=(g == 0), stop=(g == 3))
            for gg in range(4):
                dst = o_all[:, gg * 256:(gg + 1) * 256]
                if gg % 2 == 0:
                    nc.scalar.activation(dst, o_ps[gg][:],
                                         mybir.ActivationFunctionType.Identity,
                                         bias=bias_sb[:, 4 + gg:5 + gg])
                else:
                    nc.vector.tensor_scalar_add(dst, o_ps[gg][:],
                                                bias_sb[:, 4 + gg:5 + gg])
            nc.sync.dma_start(outT[:, :], o_all[:])



    nc.compile()
    return nc


def _band_mask(T):
    """Binary S^T masks [128 keys, 256 (2 kb blocks of 128)] for global
    query tile T (0..15)."""
    j = np.arange(128)[:, None]
    r = np.arange(128)[None, :]
    out = np.zeros((128, 256), np.float32)
    for kb in range(2):
        band = (j >= r) if kb == 0 else (j <= r)
        jg = T * 128 - W + kb * 128 + j
        valid = band & (jg >= 0) & (jg < S)
        out[:, kb * 128:(kb + 1) * 128] = valid
    return out


def _prep_inputs(x, Wq, bq, Wk, bk, Wv, bv, Wo, bo):
    bf = ml_dtypes.bfloat16
    f32 = np.float32

    def pack_w(Wm, scale=1.0):
        wT = np.asarray(Wm, f32).T * scale          # [512 in, 512 out]
        return np.ascontiguousarray(
            wT.reshape(4, 128, 512).transpose(1, 0, 2).reshape(128, 2048)
            .astype(bf))

    def pack_w_gmajor(Wm, scale=1.0):
        # out-group g at cols g*512, k-chunk kk at +kk*128
        wT = np.asarray(Wm, f32).T * scale
        return np.ascontiguousarray(
            wT.reshape(4, 128, 4, 128).transpose(1, 2, 0, 3).reshape(128, 2048)
            .astype(bf))

    wk4 = pack_w_gmajor(Wk)
    wq4 = pack_w_gmajor(Wq, SCALE)
    wv4 = pack_w(Wv)
    wo4 = pack_w(Wo)
    bo_eff = np.asarray(bo, f32) + np.asarray(Wo, f32) @ np.asarray(bv, f32)
    bias = np.zeros((128, 12), f32)
    bias[:, 0:4] = np.asarray(bk, f32).reshape(4, 128).T
    bias[:, 4:8] = bo_eff.reshape(4, 128).T
    bias[:, 8:12] = (np.asarray(bq, f32) * SCALE).reshape(4, 128).T

    identw = np.eye(128, dtype=f32).astype(bf)
    bias_view = np.ascontiguousarray(bias).view(bf)           # [128, 24]
    xf = np.asarray(x, f32)
    in_maps = []
    for c in range(N_CORES):
        klo = c * SEQ_PER_CORE - W
        lo, hi = max(0, klo), min(S, klo + CHUNK)
        xT_c = np.zeros((D, CHUNK), f32)
        xT_c[:, lo - klo:hi - klo] = xf[0, lo:hi, :].T
        x4 = np.ascontiguousarray(
            xT_c.reshape(4, 128, CHUNK).transpose(1, 0, 2)
            .reshape(128, 4 * CHUNK).astype(bf))
        m = np.concatenate([_band_mask(c * 2), _band_mask(c * 2 + 1)],
                           axis=1)
        m = ((m - 1.0) * 30.0).astype(bf)
        d1ac = np.ascontiguousarray(x4[:, 0:768])
        d1a2c = np.ascontiguousarray(x4[:, 768:1536])
        d1bc = np.ascontiguousarray(
            np.concatenate([wk4[:, 0:512], wq4[:, 0:512], bias_view], axis=1))
        d3c = np.ascontiguousarray(
            np.concatenate([wk4[:, 512:2048], wq4[:, 512:2048],
                            m, identw], axis=1))
        in_maps.append({
            "d1a": d1ac, "d1a2": d1a2c, "d1b": d1bc, "wv4": wv4,
            "d3": d3c, "wo4": wo4,
        })
    return in_maps


def kernel(x, Wq, bq, Wk, bk, Wv, bv, Wo, bo):
    if "nc" not in _CACHE:
        _CACHE["nc"] = _build_program()
    nc = _CACHE["nc"]
    in_maps = _prep_inputs(x, Wq, bq, Wk, bk, Wv, bv, Wo, bo)
    res = bass_utils.run_bass_kernel_spmd(nc, in_maps,
                                          core_ids=list(range(N_CORES)))
    out = np.empty((1, S, D), np.float32)
    for c in range(N_CORES):
        arr = np.asarray(res.results[c]["outT"]).astype(np.float32)
        chunk = arr.reshape(128, 4, SEQ_PER_CORE).transpose(1, 0, 2) \
                   .reshape(D, SEQ_PER_CORE).T
        out[0, c * SEQ_PER_CORE:(c + 1) * SEQ_PER_CORE, :] = chunk
    return out



# revision 34
# speedup vs baseline: 1.1588x; 1.1588x over previous
"""Locally banded sparse attention (window=64) on 8 Trainium2 NeuronCores.

Sequence-parallel: each core owns 256 contiguous query positions and
receives a 384-row x chunk (its 256 rows + 64-row halo on each side,
zero-padded at the sequence edges) plus a full replica of the four
projection matrices.  No device collectives are needed.

All matmuls run in bf16 (fp32 PSUM accumulation).  Attention scores are
computed directly in transposed layout S^T[key, query] = kT.T @ qT, and
the P@V matmul uses P^T as the stationary operand so its output lands
query-major: av[q, d] with the softmax denominator Z[q] riding along as a
ones-column of V (col 64 of each head's 65-wide slot).  Normalization is
then a per-partition reciprocal + tensor_scalar multiply — no partition
broadcasts anywhere.  The q-major attention output is PE-transposed back
to d-major for the output projection.

Host-side folds: SCALE and bq into Wq/bq, bv into an effective bo
(out += bv @ Wo.T is query-independent).

Engine balance: PE matmuls (plus HAM warm-up dummies during the input
DMA); ACT exp + kT/o copies; DVE qT/vaug copies, reciprocals,
normalization, transpose copies; GPSIMD half the band-mask multiplies.

v3 skeleton: 4 consumption-ordered input DMAs [x | wk g0 | wq g0 |
biases], [wv], [wk g1-3 | wq g1-3 | mask | ident], [wo], so K/Q group-0
projections and the V projection overlap the remaining weight traffic;
group 1-3 K/Q projections and the aT transposes interleave with the
attention steps.
"""

import numpy as np
import ml_dtypes

import concourse.bass as bass
import concourse.tile as tile
from concourse import bacc, mybir
from concourse import bass_utils

F32 = mybir.dt.float32
BF16 = mybir.dt.bfloat16
N_CORES = 8
S = 2048
D = 512
H = 8
DK = 64
W = 64
SCALE = 1.0 / np.sqrt(DK)
SEQ_PER_CORE = S // N_CORES          # 256
CHUNK = SEQ_PER_CORE + 2 * W         # 384 rows of k/v context per core

_CACHE = {}


def _build_program():
    nc = bacc.Bacc("TRN2", target_bir_lowering=False, debug=False,
                   num_devices=N_CORES)

    # d1 = x (1536) | wk g0 (512) | wq g0 (512) | biases f32-bitcast (24)
    # d3 = wk g1-3 (1536) | wq g1-3 (1536) | mask (512) | ident (128)
    d1a = nc.dram_tensor("d1a", [128, 768], BF16, kind="ExternalInput").ap()
    d1a2 = nc.dram_tensor("d1a2", [128, 768], BF16, kind="ExternalInput").ap()
    d1b = nc.dram_tensor("d1b", [128, 1048], BF16, kind="ExternalInput").ap()
    wv4 = nc.dram_tensor("wv4", [128, 2048], BF16, kind="ExternalInput").ap()
    d3 = nc.dram_tensor("d3", [128, 3712], BF16, kind="ExternalInput").ap()
    wo4 = nc.dram_tensor("wo4", [128, 2048], BF16, kind="ExternalInput").ap()
    outT = nc.dram_tensor("outT", [128, 4 * SEQ_PER_CORE], BF16,
                          kind="ExternalOutput").ap()

    with tile.TileContext(nc) as tc:
        with (
            tc.tile_pool(name="const", bufs=1) as cpool,
            tc.tile_pool(name="pp", bufs=2, space="PSUM") as pp,
            tc.tile_pool(name="s_ps", bufs=4, space="PSUM") as s_ps,
            tc.tile_pool(name="av_ps", bufs=2, space="PSUM") as av_ps,
            tc.tile_pool(name="soft", bufs=6) as soft,
            tc.tile_pool(name="small", bufs=4) as small,
        ):
            def persist(shape, tag, dtype=BF16):
                return cpool.tile(shape, dtype, tag=tag, name=tag)

            d1_sb = persist([128, 2584], "d1")
            wv_sb = persist([128, 2048], "wv")
            d3_sb = persist([128, 3712], "d3")
            ident = d3_sb[:, 3584:3712]
            wo_sb = persist([128, 2048], "wo")
            x_sb = d1_sb[:, 0:1536]
            bias_sb = d1_sb[:, 2560:2584].bitcast(F32)

            def wk_slice(g, kk):
                if g == 0:
                    return d1_sb[:, 1536 + kk * 128:1536 + kk * 128 + 128]
                return d3_sb[:, (g - 1) * 512 + kk * 128:
                             (g - 1) * 512 + kk * 128 + 128]

            def wq_slice(g, kk):
                if g == 0:
                    return d1_sb[:, 2048 + kk * 128:2048 + kk * 128 + 128]
                return d3_sb[:, 1536 + (g - 1) * 512 + kk * 128:
                             1536 + (g - 1) * 512 + kk * 128 + 128]
            mask_sb = d3_sb[:, 3072:3584]
            k_sb = [persist([128, CHUNK], f"k{g}") for g in range(4)]
            q_sb = [persist([128, SEQ_PER_CORE], f"q{g}") for g in range(4)]
            # v with a ones column per head: head h at cols [h*65, +64], Z at h*65+64
            vaug = [persist([128, 8 * 65], f"v{r}") for r in range(3)]
            aT_sb = [persist([128, D], f"aT{t}") for t in range(2)]
            a_sb = [persist([128, SEQ_PER_CORE], f"a{g}") for g in range(4)]
            o_all = persist([128, 4 * SEQ_PER_CORE], "o_all")
            scratch = persist([128, 256], "scratch")

            # input DMAs: single sync HWDGE ring, strict consumption order
            # (in-ring transfers complete FIFO; each dma_start costs ~0.6us
            # of ring issue time, so inputs are merged into 4 transfers)
            nc.sync.dma_start(d1_sb[:, 0:768], d1a[:, :])
            nc.sync.dma_start(d1_sb[:, 768:1536], d1a2[:, :])
            nc.sync.dma_start(d1_sb[:, 1536:2584], d1b[:, :])
            nc.sync.dma_start(wv_sb[:], wv4[:, :])
            nc.sync.dma_start(d3_sb[:], d3[:, :])
            nc.sync.dma_start(wo_sb[:], wo4[:, :])

            def vaug_ap(r, col0, ncols):
                base = vaug[r][:]
                p_step = base.ap[0][0]
                return bass.AP(base.tensor, base.offset + col0,
                               [[p_step, 128], [65, 8], [1, ncols]])

            for r in range(3):
                nc.vector.memset(vaug_ap(r, 64, 1), 1.0)

            # HAM warm-up: keep the PE streaming dummy matmuls while the
            # weight DMAs land so real matmuls run at 2.4 GHz, not 1.2
            nc.vector.memset(scratch[:], 0.0)
            # touch the ACT table early so its 1.3us load runs during the
            # input-DMA wait, not on the K-evac critical path
            warm_act = small.tile([128, 8], BF16, tag="wa", name="wa")
            nc.scalar.activation(warm_act[:], scratch[:, 0:8],
                                 mybir.ActivationFunctionType.Exp)
            for w in range(18):
                wps = s_ps.tile([128, 256], F32, tag="s", name="warm")
                nc.tensor.matmul(wps[:], scratch[:, 0:128], scratch[:],
                                 start=True, stop=True)

            # ---- projections -----------------------------------------
            def emit_kproj(g):
                ps = pp.tile([128, 512], F32, tag="pp", name="pp")
                for kk in range(4):
                    nc.tensor.matmul(ps[:, :CHUNK],
                                     wk_slice(g, kk),
                                     x_sb[:, kk * CHUNK:(kk + 1) * CHUNK],
                                     start=(kk == 0), stop=(kk == 3))
                nc.scalar.activation(k_sb[g][:], ps[:, :CHUNK],
                                     mybir.ActivationFunctionType.Identity,
                                     bias=bias_sb[:, g:g + 1])

            def emit_qproj(g):
                ps = pp.tile([128, 512], F32, tag="pp", name="pp")
                for kk in range(4):
                    nc.tensor.matmul(ps[:, :SEQ_PER_CORE],
                                     wq_slice(g, kk),
                                     x_sb[:, kk * CHUNK + W:
                                          kk * CHUNK + W + SEQ_PER_CORE],
                                     start=(kk == 0), stop=(kk == 3))
                nc.vector.tensor_scalar_add(q_sb[g][:], ps[:, :SEQ_PER_CORE],
                                            bias_sb[:, 8 + g:9 + g])

            emit_kproj(0)
            emit_qproj(0)
            # v natural [keys, dout] -> vaug 65-wide head slots
            for r in range(3):
                ps = pp.tile([128, 512], F32, tag="pp", name="pp")
                for kk in range(4):
                    nc.tensor.matmul(ps[:],
                                     x_sb[:, kk * CHUNK + r * 128:
                                          kk * CHUNK + r * 128 + 128],
                                     wv_sb[:, kk * 512:(kk + 1) * 512],
                                     start=(kk == 0), stop=(kk == 3))
                if r % 2 == 0:
                    nc.vector.tensor_copy(vaug_ap(r, 0, 64), ps[:])
                else:
                    nc.scalar.activation(vaug_ap(r, 0, 64), ps[:],
                                         mybir.ActivationFunctionType.Copy)

            # ---- banded attention (S^T scores, q-major AV) ----------
            # software pipeline: S^T for step i runs on PE while step i-1
            # finishes softmax on ACT/GPSIMD, then its AV matmuls issue.
            steps = [(g, t) for g in range(4) for t in range(2)]
            pend = None   # (g, t, pA, pB, avz)
            tposes = []

            def emit_transpose(g, t):
                tp = av_ps.tile([128, 128], BF16, tag="av", name="tp")
                nc.tensor.transpose(tp[:],
                                    aT_sb[t][:, g * 128:(g + 1) * 128],
                                    ident)
                if g % 2 == 0:
                    nc.vector.tensor_copy(
                        a_sb[g][:, t * 128:(t + 1) * 128], tp[:])
                else:
                    nc.scalar.activation(
                        a_sb[g][:, t * 128:(t + 1) * 128], tp[:],
                        mybir.ActivationFunctionType.Copy)

            def emit_av(st):
                g, t, pA, pB, avz = st
                hA, hB = 2 * g, 2 * g + 1
                for kb in range(2):
                    nc.tensor.matmul(avz[:, 0:65],
                                     pA[:, kb * 128:(kb + 1) * 128],
                                     vaug[t + kb][:, hA * 65:hA * 65 + 65],
                                     start=(kb == 0), stop=(kb == 1))
                for kb in range(2):
                    nc.tensor.matmul(avz[:, 65:130],
                                     pB[:, kb * 128:(kb + 1) * 128],
                                     vaug[t + kb][:, hB * 65:hB * 65 + 65],
                                     start=(kb == 0), stop=(kb == 1))
                rz2 = small.tile([128, 2], F32, tag="rz", name="rz")
                zbase = avz[:]
                pstep = zbase.ap[0][0]
                zin = bass.AP(zbase.tensor, zbase.offset + 64,
                              [[pstep, 128], [65, 2]])
                nc.vector.reciprocal_approx_fast(rz2[:], zin)
                nc.vector.tensor_scalar_mul(aT_sb[t][:, hA * 64:hA * 64 + 64],
                                            avz[:, 0:64], rz2[:, 0:1])
                nc.vector.tensor_scalar_mul(aT_sb[t][:, hB * 64:hB * 64 + 64],
                                            avz[:, 65:129], rz2[:, 1:2])

            for g, t in steps:
                sA = s_ps.tile([128, 256], F32, tag="s", name="s")
                sB = s_ps.tile([128, 256], F32, tag="s", name="s")
                for kb in range(2):
                    kc = (t + kb) * 128
                    nc.tensor.matmul(sA[:, kb * 128:(kb + 1) * 128],
                                     k_sb[g][0:64, kc:kc + 128],
                                     q_sb[g][0:64, t * 128:(t + 1) * 128],
                                     start=True, stop=True)
                    nc.tensor.matmul(sB[:, kb * 128:(kb + 1) * 128],
                                     k_sb[g][64:128, kc:kc + 128],
                                     q_sb[g][64:128, t * 128:(t + 1) * 128],
                                     start=True, stop=True)
                eA = soft.tile([128, 256], BF16, tag="eA", name="eA")
                eB = soft.tile([128, 256], BF16, tag="eB", name="eB")
                nc.scalar.activation(eA[:], sA[:],
                                     mybir.ActivationFunctionType.Exp)
                nc.scalar.activation(eB[:], sB[:],
                                     mybir.ActivationFunctionType.Exp)
                pA = soft.tile([128, 256], BF16, tag="pA", name="pA")
                pB = soft.tile([128, 256], BF16, tag="pB", name="pB")
                msk = mask_sb[:, t * 256:(t + 1) * 256]
                nc.gpsimd.tensor_mul(pA[:], eA[:], msk)
                nc.vector.tensor_mul(pB[:], eB[:], msk)
                avz = av_ps.tile([128, 130], F32, tag="av", name="av")
                if pend is not None:
                    emit_av(pend)
                    tposes.append((pend[0], pend[1]))
                pend = (g, t, pA, pB, avz)
                if t == 0 and g < 3:
                    emit_kproj(g + 1)
                    emit_qproj(g + 1)
                if len(tposes) >= 1:
                    emit_transpose(*tposes.pop(0))
            emit_av(pend)
            tposes.append((pend[0], pend[1]))
            for gt in tposes:
                emit_transpose(*gt)

            # ---- O-projection (gg-outer over 4 s_ps banks) ----------
            o_ps = [s_ps.tile([128, 256], F32, tag="s", name=f"o{gg}")
                    for gg in range(4)]
            for gg in range(4):
                for g in range(4):
                    nc.tensor.matmul(o_ps[gg][:],
                                     wo_sb[:, g * 512 + gg * 128:
                                           g * 512 + gg * 128 + 128],
                                     a_sb[g][:],
                                     start=(g == 0), stop=(g == 3))
            for gg in range(4):
                dst = o_all[:, gg * 256:(gg + 1) * 256]
                if gg % 2 == 0:
                    nc.scalar.activation(dst, o_ps[gg][:],
                                         mybir.ActivationFunctionType.Identity,
                                         bias=bias_sb[:, 4 + gg:5 + gg])
                else:
                    nc.vector.tensor_scalar_add(dst, o_ps[gg][:],
                                                bias_sb[:, 4 + gg:5 + gg])
            nc.sync.dma_start(outT[:, :], o_all[:])



    nc.compile()
    return nc


def _band_mask(T):
    """Binary S^T masks [128 keys, 256 (2 kb blocks of 128)] for global
    query tile T (0..15)."""
    j = np.arange(128)[:, None]
    r = np.arange(128)[None, :]
    out = np.zeros((128, 256), np.float32)
    for kb in range(2):
        band = (j >= r) if kb == 0 else (j <= r)
        jg = T * 128 - W + kb * 128 + j
        valid = band & (jg >= 0) & (jg < S)
        out[:, kb * 128:(kb + 1) * 128] = valid
    return out


def _prep_inputs(x, Wq, bq, Wk, bk, Wv, bv, Wo, bo):
    bf = ml_dtypes.bfloat16
    f32 = np.float32

    def pack_w(Wm, scale=1.0):
        wT = np.asarray(Wm, f32).T * scale          # [512 in, 512 out]
        return np.ascontiguousarray(
            wT.reshape(4, 128, 512).transpose(1, 0, 2).reshape(128, 2048)
            .astype(bf))

    def pack_w_gmajor(Wm, scale=1.0):
        # out-group g at cols g*512, k-chunk kk at +kk*128
        wT = np.asarray(Wm, f32).T * scale
        return np.ascontiguousarray(
            wT.reshape(4, 128, 4, 128).transpose(1, 2, 0, 3).reshape(128, 2048)
            .astype(bf))

    wk4 = pack_w_gmajor(Wk)
    wq4 = pack_w_gmajor(Wq, SCALE)
    wv4 = pack_w(Wv)
    wo4 = pack_w(Wo)
    bo_eff = np.asarray(bo, f32) + np.asarray(Wo, f32) @ np.asarray(bv, f32)
    bias = np.zeros((128, 12), f32)
    bias[:, 0:4] = np.asarray(bk, f32).reshape(4, 128).T
    bias[:, 4:8] = bo_eff.reshape(4, 128).T
    bias[:, 8:12] = (np.asarray(bq, f32) * SCALE).reshape(4, 128).T

    identw = np.eye(128, dtype=f32).astype(bf)
    bias_view = np.ascontiguousarray(bias).view(bf)           # [128, 24]
    xf = np.asarray(x, f32)
    in_maps = []
    for c in range(N_CORES):
        klo = c * SEQ_PER_CORE - W
        lo, hi = max(0, klo), min(S, klo + CHUNK)
        xT_c = np.zeros((D, CHUNK), f32)
        xT_c[:, lo - klo:hi - klo] = xf[0, lo:hi, :].T
        x4 = np.ascontiguousarray(
            xT_c.reshape(4, 128, CHUNK).transpose(1, 0, 2)
            .reshape(128, 4 * CHUNK).astype(bf))
        m = np.concatenate([_band_mask(c * 2), _band_mask(c * 2 + 1)],
                           axis=1).astype(bf)
        d1ac = np.ascontiguousarray(x4[:, 0:768])
        d1a2c = np.ascontiguousarray(x4[:, 768:1536])
        d1bc = np.ascontiguousarray(
            np.concatenate([wk4[:, 0:512], wq4[:, 0:512], bias_view], axis=1))
        d3c = np.ascontiguousarray(
            np.concatenate([wk4[:, 512:2048], wq4[:, 512:2048],
                            m, identw], axis=1))
        in_maps.append({
            "d1a": d1ac, "d1a2": d1a2c, "d1b": d1bc, "wv4": wv4,
            "d3": d3c, "wo4": wo4,
        })
    return in_maps


def kernel(x, Wq, bq, Wk, bk, Wv, bv, Wo, bo):
    if "nc" not in _CACHE:
        _CACHE["nc"] = _build_program()
    nc = _CACHE["nc"]
    in_maps = _prep_inputs(x, Wq, bq, Wk, bk, Wv, bv, Wo, bo)
    res = bass_utils.run_bass_kernel_spmd(nc, in_maps,
                                          core_ids=list(range(N_CORES)))
    out = np.empty((1, S, D), np.float32)
    for c in range(N_CORES):
        arr = np.asarray(res.results[c]["outT"]).astype(np.float32)
        chunk = arr.reshape(128, 4, SEQ_PER_CORE).transpose(1, 0, 2) \
                   .reshape(D, SEQ_PER_CORE).T
        out[0, c * SEQ_PER_CORE:(c + 1) * SEQ_PER_CORE, :] = chunk
    return out

